# revision 1
# baseline (speedup 1.0000x reference)
"""Trainium2 Bass kernel for nn_FPSWE_pool (segment_reduce / sliced-Wasserstein pool).

Math (validated against the reference to ~4e-7 rel):
    W  = theta_v / ||theta_v||_row                       # [Pd, d_in]
    Xs = X @ W.T                                         # [N, Pd]
    S[e, :, p] = sort(Xs[e*32:(e+1)*32, p])              # per-edge, per-col sort
    out[e, p]  = c[p] - (1/M) * sum_r A[r, p] * S[e, r, p]
where A [32, Pd] and c [Pd] are small matrices computed on the host from
(weight, anchors, deg) only: A folds the anchor-grid linear interpolation,
the per-column argsort of anchors, and the weight matrix; c is the
edge-independent term (weight * anchors.T).mean(axis=1).

Sharding: edges are split 8 ways (contiguous 32-row degree blocks), per the
contiguous-block structure of hyperedge_index; params replicated.

Device work per core (256 edges = 8192 rows/core):
  1. one input DMA: [consts | X.T slice] fp32 into SBUF, issued as 4
     range-chunks so compute starts as soon as the first 2 MB land
  2. fp32 TensorE matmul: Xs.T[ph, rows] = W_h @ X.T (2 proj halves of 128),
     paired into 2-bank PSUM tiles; ScalarE copies cast PSUM fp32 -> SBUF bf16
  3. flip-form bitonic sort network (15 stages) over each 32-row block along
     the free dim on VectorE, in two edge-chunks (64 + 192) per half so the
     first chunk sorts while the rest of the input is still streaming in.
     A rotate-by-1 physical slot layout keeps every stage's innermost AP
     step at +-1 so the DVE bf16 2x perf mode applies (proven minimal: one
     flip-32 stage must run strided at 1x).
  4. A-weighted reduce: bf16 mult + one bf16 pairwise add level + fp32
     tensor_reduce of 16 + affine -> out.T; one output DMA per proj half.

Hardware time: ~197 us on 8 cores (VectorE-bound; the sort is ~75% of it).
Compiler notes: walrus allows ONE sync-wait command per compute instruction,
hence the bacc.Bacc + nc.compile() path (event-semaphore conversion) and the
"gate" instructions that pre-absorb DMA waits on PE/DVE.
"""

import os
from contextlib import ExitStack

import numpy as np

E_EDGES, DEG, D_IN, N_PROJ, M_ANCH = 2048, 32, 128, 256, 128
N_CORES = 8
E_LOC = E_EDGES // N_CORES          # 256 edges per core
ROWS_LOC = E_LOC * DEG              # 8192 rows per core
PH = N_PROJ // 128                  # 2 projection halves
CONST_W = N_PROJ + 32 + PH          # wt(256) | A bf16-packed-as-f32(32) | c(2)
IN_W = CONST_W + ROWS_LOC           # full per-core input width (f32 cols)

LAST_RESULTS = None                 # test.py reads trace info from here


# ----------------------------------------------------------------- network ---
def _rot(i):
    return ((i << 1) | (i >> 4)) & 31


def _sort_stages(E):
    """Flip-form bitonic(32) stages in rotate-1 physical layout.

    Returns [(lo_off, lo_dims, hi_off, hi_dims)] over a free axis of E*32
    elements; each side covers E*16 elements, pairing elementwise in stream
    order; ascending logical order (min -> lo side).
    """
    out = []
    for m in range(1, 6):
        if m <= 4:
            lo_dims = [(1 << (m + 1), E * (1 << (4 - m)))]
            hi_dims = [(1 << (m + 1), E * (1 << (4 - m)))]
            if m >= 2:
                lo_dims.append((2, 1 << (m - 1)))
                hi_dims.append((-2, 1 << (m - 1)))
            lo_dims.append((1, 2))
            hi_dims.append((1, 2))
            out.append((0, lo_dims, (1 << (m + 1)) - 2, hi_dims))
        else:
            out.append((0, [(32, E), (2, 16)], 31, [(32, E), (-2, 16)]))
        d = (1 << m) // 4
        while d >= 1:
            f = d.bit_length()          # phys bit = k+1
            lo_dims = [(1 << (f + 1), E * (1 << (4 - f))), (1, 1 << f)]
            hi_dims = [(1 << (f + 1), E * (1 << (4 - f))), (1, 1 << f)]
            out.append((0, lo_dims, 1 << f, hi_dims))
            d //= 2
    assert len(out) == 15
    return out


# ------------------------------------------------------------- bass program ---
def _emit(tc, in_d, o_d):
    """Emit the per-core program.

    in_d [128, IN_W] f32: [ wt.T | A(bf16 packed) | c | X.T row-slice ]
    o_d  [PH, 128, E_LOC] f32: out.T per proj half
    """
    import concourse.mybir as mybir
    from concourse.ap import AP

    nc = tc.nc
    f32 = mybir.dt.float32
    bf16 = mybir.dt.bfloat16
    CH = 512                     # row-chunk per matmul
    NCH = ROWS_LOC // CH         # 16
    NDMA = 4                     # input DMA range-chunks

    with ExitStack() as ctx:
        main_p = ctx.enter_context(tc.tile_pool(name="main", bufs=1))
        ps_mm = ctx.enter_context(tc.tile_pool(name="ps_mm", bufs=3, space="PSUM"))
        sort_p = ctx.enter_context(tc.tile_pool(name="sort", bufs=1))
        out_p = ctx.enter_context(tc.tile_pool(name="out", bufs=2))

        xin = main_p.tile([128, IN_W], f32)
        step = ROWS_LOC // NDMA
        nc.sync.dma_start(xin[:, :CONST_W + step], in_d[:, :CONST_W + step])
        for g in range(1, NDMA):
            lo = CONST_W + g * step
            nc.sync.dma_start(xin[:, lo:lo + step], in_d[:, lo:lo + step])

        wt_sb = xin[:, :N_PROJ]
        a_sb = xin[:, N_PROJ:N_PROJ + 32].bitcast(bf16)      # [128, 64]
        c_sb = xin[:, N_PROJ + 32:N_PROJ + 32 + PH]
        xt = xin[:, CONST_W:]                                # [128, ROWS_LOC]

        # B0/B1 hold Xs.T bf16 per proj half; C0/C1 are sort ping-pong scratch.
        B = [sort_p.tile([128, ROWS_LOC], bf16, tag=f"B{h}", name=f"B{h}")
             for h in range(PH)]
        C = [sort_p.tile([128, ROWS_LOC], bf16, tag=f"C{h}", name=f"C{h}")
             for h in range(PH)]

        # Walrus allows only ONE sync-wait command on a Matmult (LDW struct).
        # Matmuls at DMA-group seams would need two (new DMA range + PSUM
        # slot release), so a tiny "gate" matmul absorbs each group's DMA
        # wait first; the real matmuls then only wait on the ACT copy.
        from concourse.bass import _add_dep_helper
        ps_gate = ctx.enter_context(tc.tile_pool(name="ps_gate", bufs=1, space="PSUM"))
        gates = {}

        def emit_gate(g):
            pg = ps_gate.tile([128, 1], f32, tag="g", name=f"gate{g}")
            gates[g] = nc.tensor.matmul(
                pg[:], wt_sb[:, :128],
                xt[:, g * (ROWS_LOC // NDMA):g * (ROWS_LOC // NDMA) + 1],
                start=True, stop=True,
            )

        # Same trick for the DVE: its first consts-range read (the A-broadcast
        # mult) must not carry both a DMA wait and a self wait (TT struct also
        # caps at one sync-wait command).
        dve_gate_t = out_p.tile([128, 2], f32, name="dve_gate_t")
        dve_gate = nc.vector.tensor_copy(dve_gate_t[:], c_sb[:, :PH])

        # HAM warm-up: the PE clock gate needs ~3.4us of sustained activity to
        # lift the cold 1.2 GHz throttle.  Dummy matmuls on a zeroed scratch
        # tile during the preamble+DMA window make the first real (critical-
        # path) matmuls run at 2.4 GHz.
        warm_t = out_p.tile([128, 128], f32, name="warm_t")
        nc.vector.memset(warm_t[:], 0.0)
        pw = ps_gate.tile([128, 128], f32, tag="w", name="warm_ps")
        for _ in range(16):
            nc.tensor.matmul(pw[:], warm_t[:], warm_t[:], start=True, stop=True)

        grp = NCH // NDMA
        for h in range(PH):
            for jj in range(NCH // 2):
                pmm = ps_mm.tile([128, 2 * CH], f32, tag="mm", name=f"mm{h}_{jj}")
                for k in range(2):
                    j = 2 * jj + k
                    g = j // grp
                    if g >= 1 and g not in gates:
                        emit_gate(g)
                    mm = nc.tensor.matmul(
                        pmm[:, k * CH:(k + 1) * CH],
                        wt_sb[:, h * 128:(h + 1) * 128],
                        xt[:, j * CH:(j + 1) * CH],
                        start=True, stop=True,
                    )
                    if g >= 1:
                        _add_dep_helper(
                            mm.ins, gates[g].ins, sync=False,
                            reason="order mm after its DMA-group gate",
                        )
                nc.scalar.copy(
                    B[h][:, jj * 2 * CH:(jj + 1) * 2 * CH], pmm[:]
                )

        # h0 is chunked so sorting starts while the input still streams in;
        # h1's data is ready long before the DVE reaches it, so full width
        # avoids the per-op overhead of extra chunks.
        CHUNKS_H = {0: [64, 192], 1: [E_LOC]}
        alu_min = mybir.AluOpType.min
        alu_max = mybir.AluOpType.max

        def side_ap(tile, off, dims):
            base = tile[:]
            return AP(
                tensor=base.tensor,
                offset=base.offset + off,
                ap=[list(base.ap[0])] + [[s, c] for (s, c) in dims],
            )

        otile = out_p.tile([128, PH * E_LOC], f32, name="otile")

        for h in range(PH):
            eoff = 0
            for cch, ECE in enumerate(CHUNKS_H[h]):
                co = eoff * DEG
                stages = _sort_stages(ECE)
                cur, oth = B[h], C[h]
                first_tt = (h == 0 and cch == 0)
                for (lo_off, lo_dims, hi_off, hi_dims) in stages:
                    for op, w_off, w_dims in (
                        (alu_min, lo_off, lo_dims),
                        (alu_max, hi_off, hi_dims),
                    ):
                        tt = nc.vector.tensor_tensor(
                            out=side_ap(oth, co + w_off, w_dims),
                            in0=side_ap(cur, co + lo_off, lo_dims),
                            in1=side_ap(cur, co + hi_off, hi_dims),
                            op=op,
                        )
                        if first_tt:
                            _add_dep_helper(
                                tt.ins, dve_gate.ins, sync=False,
                                reason="order sort after DVE consts gate",
                            )
                            first_tt = False
                    cur, oth = oth, cur

                # cur holds the sorted chunk; oth is scratch.
                a_h = a_sb[:, h * DEG:(h + 1) * DEG].unsqueeze(1).broadcast_to(
                    [128, ECE, DEG]
                )
                nc.vector.tensor_tensor(
                    out=side_ap(oth, co, [(DEG, ECE), (1, DEG)]),
                    in0=side_ap(cur, co, [(DEG, ECE), (1, DEG)]),
                    in1=a_h,
                    op=mybir.AluOpType.mult,
                )
                # level-1 pairwise add (bf16, in place), then fp32 reduce of 16
                lo16 = side_ap(oth, co, [(DEG, ECE), (1, 16)])
                nc.vector.tensor_tensor(
                    out=lo16, in0=lo16,
                    in1=side_ap(oth, co + 16, [(DEG, ECE), (1, 16)]),
                    op=mybir.AluOpType.add,
                )
                osl = otile[:, h * E_LOC + eoff:h * E_LOC + eoff + ECE]
                nc.vector.tensor_reduce(
                    out=osl, in_=lo16,
                    axis=mybir.AxisListType.X, op=mybir.AluOpType.add,
                )
                nc.vector.tensor_scalar(
                    out=osl, in0=osl,
                    scalar1=-1.0 / M_ANCH, scalar2=c_sb[:, h:h + 1],
                    op0=mybir.AluOpType.mult, op1=mybir.AluOpType.add,
                )
                eoff += ECE

        for h in range(PH):
            nc.sync.dma_start(o_d[h], otile[:, h * E_LOC:(h + 1) * E_LOC])


def _build():
    import concourse.bacc as bacc
    import concourse.mybir as mybir
    import concourse.tile as tile

    nc = bacc.Bacc(
        "TRN2", target_bir_lowering=False, debug=False,
        enable_asserts=False, num_devices=N_CORES,
    )
    f32 = mybir.dt.float32
    in_d = nc.dram_tensor("xtc", [128, IN_W], f32, kind="ExternalInput").ap()
    o_d = nc.dram_tensor(
        "o", [PH, 128, E_LOC], f32, kind="ExternalOutput"
    ).ap()
    with tile.TileContext(nc) as tc:
        _emit(tc, in_d, o_d)
    nc.compile()
    return nc


_CACHE = {}


def _host_consts(theta_v, weight, anchors):
    import ml_dtypes

    W = theta_v / np.linalg.norm(theta_v, axis=1, keepdims=True)
    u = np.linspace(0.0, 1.0, M_ANCH, dtype=np.float32) * np.float32(0.99998)
    a = u * np.float32(DEG - 1.0) / np.float32(0.99999)
    r0 = np.clip(np.floor(a), 0.0, DEG - 2.0)
    frac = (a - r0).astype(np.float32)
    r0 = r0.astype(np.int64)
    Rind = np.argsort(anchors, axis=0, kind="stable")          # [M, Pd]
    wperm = np.zeros((M_ANCH, N_PROJ), np.float32)
    np.put_along_axis(wperm, Rind, weight.T, axis=0)
    A = np.zeros((DEG, N_PROJ), np.float32)
    np.add.at(A, r0, wperm * (1.0 - frac)[:, None])
    np.add.at(A, r0 + 1, wperm * frac[:, None])
    c = (weight * anchors.T).mean(axis=1).astype(np.float32)   # [Pd]

    # physical slot layout: rank r lives at slot rot(r)
    A_phys = np.zeros_like(A)
    for r in range(DEG):
        A_phys[_rot(r)] = A[r]
    A2 = np.zeros((128, PH * DEG), np.float32)
    c2 = np.zeros((128, PH), np.float32)
    for h in range(PH):
        A2[:, h * DEG:(h + 1) * DEG] = A_phys[:, h * 128:(h + 1) * 128].T
        c2[:, h] = c[h * 128:(h + 1) * 128]
    A2_packed = (
        A2.astype(ml_dtypes.bfloat16).view(np.uint16)
        .reshape(128, PH * DEG).view(np.uint32).view(np.float32)
    )                                                          # [128, 32]
    consts = np.zeros((128, CONST_W), np.float32)
    consts[:, :N_PROJ] = np.ascontiguousarray(W.T, dtype=np.float32)
    consts[:, N_PROJ:N_PROJ + 32] = A2_packed
    consts[:, N_PROJ + 32:N_PROJ + 32 + PH] = c2
    return consts


def kernel(X, hyperedge_index, theta_v, weight, anchors, num_edges):
    global LAST_RESULTS
    from concourse.bass_utils import run_bass_kernel_spmd

    X = np.asarray(X, dtype=np.float32)
    theta_v = np.asarray(theta_v, dtype=np.float32)
    weight = np.asarray(weight, dtype=np.float32)
    anchors = np.asarray(anchors, dtype=np.float32)

    consts = _host_consts(theta_v, weight, anchors)
    XT = np.ascontiguousarray(X.T)                             # [128, N]
    if "nc" not in _CACHE:
        _CACHE["nc"] = _build()
    nc = _CACHE["nc"]

    in_maps = []
    for cid in range(N_CORES):
        xtc = np.empty((128, IN_W), np.float32)
        xtc[:, :CONST_W] = consts
        xtc[:, CONST_W:] = XT[:, cid * ROWS_LOC:(cid + 1) * ROWS_LOC]
        in_maps.append({"xtc": xtc})
    res = run_bass_kernel_spmd(
        nc, in_maps, core_ids=list(range(N_CORES)),
        trace=bool(int(os.environ.get("KERNEL_TRACE", "0"))),
    )
    LAST_RESULTS = res

    outT = np.empty((N_PROJ, E_EDGES), np.float32)
    for cid in range(N_CORES):
        o = res.results[cid]["o"]                    # [PH, 128, E_LOC]
        outT[:, cid * E_LOC:(cid + 1) * E_LOC] = o.reshape(N_PROJ, E_LOC)
    return np.ascontiguousarray(outT.T)



# revision 4
# speedup vs baseline: 1.4071x; 1.4071x over previous
"""Trainium2 Bass kernel for nn_FPSWE_pool (segment_reduce / sliced-Wasserstein pool).

Math (validated against the reference):
    W  = theta_v / ||theta_v||_row                       # [Pd, d_in]
    Xs = X @ W.T                                         # [N, Pd]
    S[e, :, p] = sort(Xs[e*32:(e+1)*32, p])              # per-edge, per-col sort
    out[e, p]  = c[p] - (1/M) * sum_r A[r, p] * S[e, r, p]
where A [32, Pd] and c [Pd] are computed on the host from (weight, anchors,
deg): A folds the anchor-grid interpolation and the weight matrix; c is the
edge-independent term.

Sharding: edges split 8 ways (contiguous 32-row blocks); params replicated.

Device pipeline per core (256 edges = 8192 rows):
  1. input DMA: [consts f32 | X.T bf16] in 8 range-chunks
  2. bf16 TensorE matmul -> PSUM f32 -> ScalarE copy -> SBUF bf16 (Xs.T)
  3. flip-form bitonic sort (15 stages) per 32-block along the free dim on
     VectorE in rotate-1 physical slot layout (every stage's innermost AP
     step is +-1 so the DVE bf16 2x mode applies).  Optionally (USE_FUSED)
     most stages run as single fused compare-exchange instructions via a
     custom DVE micro-op program (4 elems/cycle) installed over the
     tensor_scalar opcode-table row.
  4. A-weighted reduce: bf16 mult + two bf16 pairwise-add levels + fp32
     tensor_reduce of 8 + ScalarE affine (x * -1/M + c) -> out.T
"""

import os
from contextlib import ExitStack

import numpy as np

E_EDGES, DEG, D_IN, N_PROJ, M_ANCH = 2048, 32, 128, 256, 128
N_CORES = 8
E_LOC = E_EDGES // N_CORES          # 256 edges per core
ROWS_LOC = E_LOC * DEG              # 8192 rows per core
PH = N_PROJ // 128                  # 2 projection halves
CONST_W = N_PROJ + 32 + PH          # wt(256) | A bf16-packed-as-f32(32) | c(2)
XW_HALF = ROWS_LOC // 2             # bf16 X.T packed as f32 columns
IN_W = CONST_W + XW_HALF            # full per-core input width (f32 cols)

USE_FUSED = bool(int(os.environ.get("KERNEL_FUSED", "0")))

LAST_RESULTS = None                 # test.py reads trace info from here


# ----------------------------------------------------------------- network ---
def _rot(i):
    return ((i << 1) | (i >> 4)) & 31


def _sort_stages(E):
    """Flip-form bitonic(32) stages in rotate-1 physical layout.

    Returns [(lo_off, lo_dims, hi_off, hi_dims)] over a free axis of E*32
    elements; each side covers E*16 elements, pairing elementwise in stream
    order; ascending logical order (min -> lo side).
    """
    out = []
    for m in range(1, 6):
        if m <= 4:
            lo_dims = [(1 << (m + 1), E * (1 << (4 - m)))]
            hi_dims = [(1 << (m + 1), E * (1 << (4 - m)))]
            if m >= 2:
                lo_dims.append((2, 1 << (m - 1)))
                hi_dims.append((-2, 1 << (m - 1)))
            lo_dims.append((1, 2))
            hi_dims.append((1, 2))
            out.append((0, lo_dims, (1 << (m + 1)) - 2, hi_dims))
        else:
            out.append((0, [(32, E), (2, 16)], 31, [(32, E), (-2, 16)]))
        d = (1 << m) // 4
        while d >= 1:
            f = d.bit_length()          # phys bit = k+1
            lo_dims = [(1 << (f + 1), E * (1 << (4 - f))), (1, 1 << f)]
            hi_dims = [(1 << (f + 1), E * (1 << (4 - f))), (1, 1 << f)]
            out.append((0, lo_dims, 1 << f, hi_dims))
            d //= 2
    assert len(out) == 15
    return out


def _pa_families(E):
    """Fused-stage decomposition.

    The DVE 2-port single-src modes split the AP stream in half across the
    two read ports (port0 = first half, port1 = second half, lockstep), and
    the fused cmpx program writes the min word at port0's position and the
    max word at port1's.  So a compare-exchange stage is ONE instruction
    whose in/out AP is the lo view concatenated with the hi view:
    dims = [(hi_off - lo_off, 2), *lo_dims]  (requires hi_dims == lo_dims).
    Flip stages (reversed hi) decompose into word-pair families.

    Returns, per stage, None (use stock min/max pair) or [(off, dims), ...].
    """
    fams = []
    for m in range(1, 6):
        if m == 1:
            fams.append([(0, [(2, 2), (4, E * 8), (1, 2)])])
        elif m <= 4:
            blk = 1 << (m + 1)
            hi_off = blk - 2
            sub = []
            for k in range(1 << (m - 1)):
                delta = hi_off - 4 * k
                sub.append((2 * k, [(delta, 2), (blk, E * (32 // blk)),
                                    (1, 2)]))
            fams.append(sub)
        else:
            fams.append(None)           # m=5 flip: cross-slot, not PA-able
        d = (1 << m) // 4
        while d >= 1:
            f = d.bit_length()
            blk = 1 << (f + 1)
            fams.append([(0, [(1 << f, 2), (blk, E * (32 // blk)),
                              (1, 1 << f)])])
            d //= 2
    assert len(fams) == 15
    return fams


# ------------------------------------------------------------- bass program ---
def _emit(tc, in_d, o_d):
    """Emit the per-core program.

    in_d [128, IN_W] f32: [ wt.T bf16-packed | A bf16-packed | c | X.T bf16-packed ]
    o_d  [PH, 128, E_LOC] f32: out.T per proj half
    """
    import concourse.mybir as mybir
    from concourse.ap import AP

    nc = tc.nc
    f32 = mybir.dt.float32
    bf16 = mybir.dt.bfloat16
    CH = 512                     # row-chunk per matmul
    NCH = ROWS_LOC // CH         # 16
    NDMA = 8                     # input DMA range-chunks

    with ExitStack() as ctx:
        main_p = ctx.enter_context(tc.tile_pool(name="main", bufs=1))
        ps_mm = ctx.enter_context(tc.tile_pool(name="ps_mm", bufs=3, space="PSUM"))
        sort_p = ctx.enter_context(tc.tile_pool(name="sort", bufs=1))
        out_p = ctx.enter_context(tc.tile_pool(name="out", bufs=2))

        xin = main_p.tile([128, IN_W], f32)
        step = XW_HALF // NDMA
        nc.sync.dma_start(xin[:, :CONST_W + step], in_d[:, :CONST_W + step])
        for g in range(1, NDMA):
            lo = CONST_W + g * step
            nc.sync.dma_start(xin[:, lo:lo + step], in_d[:, lo:lo + step])

        wt_sb = xin[:, :N_PROJ].bitcast(bf16)                # [128, 512]
        a_sb = xin[:, N_PROJ:N_PROJ + 32].bitcast(bf16)      # [128, 64]
        c_sb = xin[:, N_PROJ + 32:N_PROJ + 32 + PH]
        xt = xin[:, CONST_W:].bitcast(bf16)                  # [128, ROWS_LOC]

        # B0/B1 hold Xs.T bf16 per proj half; C0/C1 are sort ping-pong scratch.
        B = [sort_p.tile([128, ROWS_LOC], bf16, tag=f"B{h}", name=f"B{h}")
             for h in range(PH)]
        C = [sort_p.tile([128, ROWS_LOC], bf16, tag=f"C{h}", name=f"C{h}")
             for h in range(PH)]

        # Walrus allows only ONE sync-wait command on a Matmult (LDW struct).
        # Matmuls at DMA-group seams would need two, so a tiny "gate" matmul
        # absorbs each group's DMA wait first.
        from concourse.bass import _add_dep_helper
        ps_gate = ctx.enter_context(tc.tile_pool(name="ps_gate", bufs=1, space="PSUM"))
        gates = {}

        def emit_gate(g):
            pg = ps_gate.tile([128, 1], f32, tag="g", name=f"gate{g}")
            gates[g] = nc.tensor.matmul(
                pg[:], wt_sb[:, :128],
                xt[:, g * (ROWS_LOC // NDMA):g * (ROWS_LOC // NDMA) + 1],
                start=True, stop=True,
            )

        # DVE consts gate: absorb the consts-DMA wait on the Vector queue
        # before the first sort op (tensor_reduce row is not hijacked).
        dve_gate_t = out_p.tile([128, 1], f32, name="dve_gate_t")
        dve_gate = nc.vector.tensor_reduce(
            out=dve_gate_t[:], in_=c_sb, axis=mybir.AxisListType.X,
            op=mybir.AluOpType.max,
        )

        # HAM warm-up for the PE clock gate.
        warm_t = out_p.tile([128, 128], f32, name="warm_t")
        nc.vector.memset(warm_t[:], 0.0)
        pw = ps_gate.tile([128, 128], f32, tag="w", name="warm_ps")
        for _ in range(16):
            nc.tensor.matmul(pw[:], warm_t[:], warm_t[:], start=True, stop=True)

        grp = NCH // NDMA            # matmul chunks per DMA group (=2)
        for h in range(PH):
            for jj in range(NCH // 2):
                pmm = ps_mm.tile([128, 2 * CH], f32, tag="mm", name=f"mm{h}_{jj}")
                for k in range(2):
                    j = 2 * jj + k
                    g = j // grp
                    if g >= 1 and g not in gates:
                        emit_gate(g)
                    mm = nc.tensor.matmul(
                        pmm[:, k * CH:(k + 1) * CH],
                        wt_sb[:, h * 128:(h + 1) * 128],
                        xt[:, j * CH:(j + 1) * CH],
                        start=True, stop=True,
                    )
                    if g >= 1:
                        _add_dep_helper(
                            mm.ins, gates[g].ins, sync=False,
                            reason="order mm after its DMA-group gate",
                        )
                nc.scalar.copy(
                    B[h][:, jj * 2 * CH:(jj + 1) * 2 * CH], pmm[:]
                )

        # h0 chunked so sorting starts while the input still streams in.
        CHUNKS_H = {0: [32, 96, 128], 1: [E_LOC]}
        alu_min = mybir.AluOpType.min
        alu_max = mybir.AluOpType.max

        def side_ap(tile, off, dims):
            base = tile[:]
            return AP(
                tensor=base.tensor,
                offset=base.offset + off,
                ap=[list(base.ap[0])] + [[s, c] for (s, c) in dims],
            )

        otile = out_p.tile([128, PH * E_LOC], f32, name="otile")

        for h in range(PH):
            eoff = 0
            for cch, ECE in enumerate(CHUNKS_H[h]):
                co = eoff * DEG
                stages = _sort_stages(ECE)
                fams = _pa_families(ECE) if USE_FUSED else [None] * 15
                cur, oth = B[h], C[h]
                first_op = (h == 0 and cch == 0)
                for si, (lo_off, lo_dims, hi_off, hi_dims) in enumerate(stages):
                    if fams[si] is not None:
                        for (foff, fdims) in fams[si]:
                            ins = nc.vector.tensor_scalar(
                                out=side_ap(oth, co + foff, fdims),
                                in0=side_ap(cur, co + foff, fdims),
                                scalar1=0.73, scalar2=None,
                                op0=mybir.AluOpType.mult,
                            )
                            if first_op:
                                _add_dep_helper(
                                    ins.ins, dve_gate.ins, sync=False,
                                    reason="order sort after DVE consts gate",
                                )
                                first_op = False
                    else:
                        for op, w_off, w_dims in (
                            (alu_min, lo_off, lo_dims),
                            (alu_max, hi_off, hi_dims),
                        ):
                            tt = nc.vector.tensor_tensor(
                                out=side_ap(oth, co + w_off, w_dims),
                                in0=side_ap(cur, co + lo_off, lo_dims),
                                in1=side_ap(cur, co + hi_off, hi_dims),
                                op=op,
                            )
                            if first_op:
                                _add_dep_helper(
                                    tt.ins, dve_gate.ins, sync=False,
                                    reason="order sort after DVE consts gate",
                                )
                                first_op = False
                    cur, oth = oth, cur

                # cur holds the sorted chunk; oth is scratch.
                a_h = a_sb[:, h * DEG:(h + 1) * DEG].unsqueeze(1).broadcast_to(
                    [128, ECE, DEG]
                )
                nc.vector.tensor_tensor(
                    out=side_ap(oth, co, [(DEG, ECE), (1, DEG)]),
                    in0=side_ap(cur, co, [(DEG, ECE), (1, DEG)]),
                    in1=a_h,
                    op=mybir.AluOpType.mult,
                )
                # two bf16 pairwise-add levels, then fp32 reduce of 8
                lo16 = side_ap(oth, co, [(DEG, ECE), (1, 16)])
                nc.vector.tensor_tensor(
                    out=lo16, in0=lo16,
                    in1=side_ap(oth, co + 16, [(DEG, ECE), (1, 16)]),
                    op=mybir.AluOpType.add,
                )
                lo8 = side_ap(oth, co, [(DEG, ECE), (1, 8)])
                nc.vector.tensor_tensor(
                    out=lo8, in0=lo8,
                    in1=side_ap(oth, co + 8, [(DEG, ECE), (1, 8)]),
                    op=mybir.AluOpType.add,
                )
                osl = otile[:, h * E_LOC + eoff:h * E_LOC + eoff + ECE]
                nc.vector.tensor_reduce(
                    out=osl, in_=lo8,
                    axis=mybir.AxisListType.X, op=mybir.AluOpType.add,
                )
                # affine (x * -1/M + c) on the Scalar engine, off the DVE
                nc.scalar.activation(
                    out=osl, in_=osl,
                    func=mybir.ActivationFunctionType.Identity,
                    bias=c_sb[:, h:h + 1], scale=-1.0 / M_ANCH,
                )
                eoff += ECE

        for h in range(PH):
            nc.sync.dma_start(o_d[h], otile[:, h * E_LOC:(h + 1) * E_LOC])


def _register_fused_ops():
    """Install the fused compare-exchange micro-op program over DVE opcode
    row 0x43 (TENSOR_SCALAR_ARITH_OP — every nc.vector.tensor_scalar variant
    dispatches there, and this kernel emits no real tensor_scalar).

    Program (4X_2PORT slot): per cycle port0 delivers one 32b word of the
    stream's first half (SRC_0/SRC_0_HI, converted to fp32), port1 one word
    of the second half (SRC_1/SRC_1_HI).  Computes elementwise min/max across
    the ports and writes the min word back at port0's position, the max word
    at port1's.  REGULAR/2X slots are poison constants: if the engine ever
    falls back, the rel-err check fails loudly rather than silently.
    """
    from concourse.dve_uop import (
        AluInp, AluOp, DveOpSpec, InpSel, OutPath, OutSel, Trigger, UopConfig,
    )
    import concourse.dve_ops as dve_ops

    def base_uop(two_port):
        u = UopConfig()
        u.enable_input(InpSel.SRC_0, 0)
        u.enable_input(InpSel.SRC_0_HI, 1)
        u.enable_input(InpSel.SRC_1, 2)
        u.enable_input(InpSel.SRC_1_HI, 3)
        u.require_inp0 = 1
        u.require_inp1 = 1 if two_port else 0
        u.trigger = (Trigger.SRC_TENSOR_DONE, Trigger.NONE, Trigger.NONE)
        u.enable_rev_ops = 1
        return u

    def prog_pa_4x():
        u = base_uop(True)
        dp = u.datapath_config
        dp[0].enable_alu(AluOp.BYPASS, AluInp.PREV_ALU_OUT)          # S0
        dp[0].pass_through_delay(0, 1, 2)
        dp[1].enable_alu(AluOp.MIN, AluInp.PREV_ALU_OUT, AluInp.PREV_DELAY_1)
        dp[1].enable_delay_from_src(AluInp.PREV_ALU_OUT, 3)          # S0 copy
        dp[1].pass_through_delay(0, 1, 2)
        dp[2].enable_alu(AluOp.MIN, AluInp.PREV_DELAY_0, AluInp.PREV_DELAY_2)
        dp[2].enable_delay_from_src(AluInp.PREV_ALU_OUT, 4)          # min0
        dp[2].pass_through_delay(0, 1, 2, 3)
        dp[3].enable_alu(AluOp.MAX, AluInp.PREV_DELAY_3, AluInp.PREV_DELAY_1)
        dp[3].enable_delay_from_src(AluInp.PREV_ALU_OUT, 5)          # min1
        dp[3].pass_through_delay(0, 2, 4)
        dp[4].enable_alu(AluOp.MAX, AluInp.PREV_DELAY_0, AluInp.PREV_DELAY_2)
        dp[4].enable_delay_from_src(AluInp.PREV_ALU_OUT, 0)          # max0
        dp[4].pass_through_delay(4, 5)
        dp[5].enable_delay_from_src(AluInp.PREV_ALU_OUT, 1)          # max1
        dp[5].pass_through_delay(0, 4, 5)
        dp[6].pass_through_delay(0, 1, 4, 5)
        dp[7].pass_through_delay(0, 1, 4, 5)
        u.enable_output(OutSel.DELAY_4, OutPath.WR0_LO)   # min0
        u.enable_output(OutSel.DELAY_5, OutPath.WR0_HI)   # min1
        u.enable_output(OutSel.DELAY_0, OutPath.WR1_LO)   # max0
        u.enable_output(OutSel.DELAY_1, OutPath.WR1_HI)   # max1
        return u

    def prog_poison(value, two_port):
        u = base_uop(two_port)
        u.inp[1] = InpSel.ZERO if value == 0.0 else InpSel.ONE_F32
        u.inp[2] = InpSel.ONE_F32
        dp = u.datapath_config
        if value == 2.0:
            dp[0].enable_alu(AluOp.ADD, AluInp.PREV_DELAY_0, AluInp.PREV_DELAY_1)
        else:
            dp[0].enable_alu(AluOp.BYPASS, AluInp.PREV_DELAY_0)
        for b in range(1, 8):
            dp[b].pass_through_alu()
        u.enable_output(OutSel.ALU_OUT, OutPath.WR0_LO)
        u.enable_output(OutSel.ALU_OUT, OutPath.WR0_HI)
        if two_port:
            u.enable_output(OutSel.ALU_OUT, OutPath.WR1_LO)
            u.enable_output(OutSel.ALU_OUT, OutPath.WR1_HI)
        return u

    spec = DveOpSpec(
        name="ANT_CMPX_PA", opcode=0x43,
        uops=[prog_poison(0.0, False)],
        uops_2x=[prog_poison(1.0, False)],
        uops_2x_2p=[prog_poison(2.0, True)],
        uops_4x=[prog_pa_4x()],
        perf_max=3, rd1_en=False,
    )
    spec.validate("v3")

    class _HandOp:
        def __init__(self, name, s):
            self.name = name
            self._spec = s
            self.subdim = False
            self.uops_sha = {}
            self.spec = None

        def compile(self, ver):
            return self._spec

    if not any(getattr(o, "name", "") == "ANT_CMPX_PA" for o in dve_ops.OPS):
        dve_ops.OPS.append(_HandOp("ANT_CMPX_PA", spec))
    return ["ANT_CMPX_PA"]


_CACHE = {}


def _build():
    import concourse.bacc as bacc
    import concourse.mybir as mybir
    import concourse.tile as tile

    nc = bacc.Bacc(
        "TRN2", target_bir_lowering=False, debug=False,
        enable_asserts=False, num_devices=N_CORES,
    )
    f32 = mybir.dt.float32
    in_d = nc.dram_tensor("xtc", [128, IN_W], f32, kind="ExternalInput").ap()
    o_d = nc.dram_tensor(
        "o", [PH, 128, E_LOC], f32, kind="ExternalOutput"
    ).ap()
    with tile.TileContext(nc) as tc:
        _emit(tc, in_d, o_d)
    if USE_FUSED:
        names = _register_fused_ops()
        nc.m.ant_custom_dve_ops = sorted(names)
    nc.compile()
    return nc


def _host_consts(theta_v, weight, anchors):
    import ml_dtypes

    W = theta_v / np.linalg.norm(theta_v, axis=1, keepdims=True)
    u = np.linspace(0.0, 1.0, M_ANCH, dtype=np.float32) * np.float32(0.99998)
    a = u * np.float32(DEG - 1.0) / np.float32(0.99999)
    r0 = np.clip(np.floor(a), 0.0, DEG - 2.0)
    frac = (a - r0).astype(np.float32)
    r0 = r0.astype(np.int64)
    Rind = np.argsort(anchors, axis=0, kind="stable")          # [M, Pd]
    wperm = np.zeros((M_ANCH, N_PROJ), np.float32)
    np.put_along_axis(wperm, Rind, weight.T, axis=0)
    A = np.zeros((DEG, N_PROJ), np.float32)
    np.add.at(A, r0, wperm * (1.0 - frac)[:, None])
    np.add.at(A, r0 + 1, wperm * frac[:, None])
    c = (weight * anchors.T).mean(axis=1).astype(np.float32)   # [Pd]

    # physical slot layout: rank r lives at slot rot(r)
    A_phys = np.zeros_like(A)
    for r in range(DEG):
        A_phys[_rot(r)] = A[r]
    A2 = np.zeros((128, PH * DEG), np.float32)
    c2 = np.zeros((128, PH), np.float32)
    for h in range(PH):
        A2[:, h * DEG:(h + 1) * DEG] = A_phys[:, h * 128:(h + 1) * 128].T
        c2[:, h] = c[h * 128:(h + 1) * 128]
    A2_packed = (
        A2.astype(ml_dtypes.bfloat16).view(np.uint16)
        .reshape(128, PH * DEG).view(np.uint32).view(np.float32)
    )                                                          # [128, 32]
    WT = np.ascontiguousarray(W.T, dtype=np.float32)           # [128, 256]
    WT_packed = (
        WT.astype(ml_dtypes.bfloat16).view(np.uint16)
        .view(np.uint32).view(np.float32)
    )                                                          # [128, 128]
    consts = np.zeros((128, CONST_W), np.float32)
    consts[:, :N_PROJ // 2] = WT_packed
    consts[:, N_PROJ:N_PROJ + 32] = A2_packed
    consts[:, N_PROJ + 32:N_PROJ + 32 + PH] = c2
    return consts


def kernel(X, hyperedge_index, theta_v, weight, anchors, num_edges):
    global LAST_RESULTS
    import ml_dtypes
    from concourse.bass_utils import run_bass_kernel_spmd

    X = np.asarray(X, dtype=np.float32)
    theta_v = np.asarray(theta_v, dtype=np.float32)
    weight = np.asarray(weight, dtype=np.float32)
    anchors = np.asarray(anchors, dtype=np.float32)

    consts = _host_consts(theta_v, weight, anchors)
    XTb = np.ascontiguousarray(X.T).astype(ml_dtypes.bfloat16)   # [128, N]
    XTp = XTb.view(np.uint16).view(np.uint32).view(np.float32)   # [128, N/2]
    if "nc" not in _CACHE:
        _CACHE["nc"] = _build()
    nc = _CACHE["nc"]

    in_maps = []
    for cid in range(N_CORES):
        xtc = np.empty((128, IN_W), np.float32)
        xtc[:, :CONST_W] = consts
        xtc[:, CONST_W:] = XTp[:, cid * XW_HALF:(cid + 1) * XW_HALF]
        in_maps.append({"xtc": xtc})
    res = run_bass_kernel_spmd(
        nc, in_maps, core_ids=list(range(N_CORES)),
        trace=bool(int(os.environ.get("KERNEL_TRACE", "0"))),
    )
    LAST_RESULTS = res

    outT = np.empty((N_PROJ, E_EDGES), np.float32)
    for cid in range(N_CORES):
        o = res.results[cid]["o"]                    # [PH, 128, E_LOC]
        outT[:, cid * E_LOC:(cid + 1) * E_LOC] = o.reshape(N_PROJ, E_LOC)
    return np.ascontiguousarray(outT.T)
